# revision 31
# baseline (speedup 1.0000x reference)
"""GraphSAGE-style pooling aggregator kernel for Trainium2 (8 NeuronCores).

Computes, for full inputs:
    h      = relu(neighbor_features @ w_pool + bias_pool)   # (n*k, dim)
    pooled = max(h.reshape(n, k, dim), axis=1)              # (n, dim)
    out    = relu(concat([src, pooled], -1) @ w + bias)     # (n, out)

Sharding: data-parallel over nodes. Core c gets nodes [c*1250, (c+1)*1250)
and the matching 40000 neighbor rows; weights replicated. No collectives.

Design (memory-regime), measured 57.1-58.3us vs 87.0us baseline:
- Host-side: neighbor/src features transposed AND cast to bf16 (HBM load
  20.5 MB -> 10.2 MB per core). Weights bf16.
- Phase 1: w_pool stationary on the PE (redundant LDWEIGHTS stripped after
  Tile legalization so matmuls run back-to-back), z^T in PSUM fp32.
- The k=32 max drains 40000 PSUM elems/partition through the only two
  engines with PSUM read ports (ACT 1.2GHz, DVE 0.96GHz). Each 128-node
  group splits INTERNALLY so both engines drain in parallel:
    nodes [0:96):   2x 1536-col PSUM tiles -> ACT relu(z+bias) copy to
                    SBUF bf16, then a 5-level DVE tensor_max tree (2x_1P).
    nodes [96:128): 2x 512-col PSUM tiles -> DVE reduce_max direct (1x),
                    then one DVE scalar_tensor_tensor (raw+bias) max 0.
  (max_k relu(z_k + b) == relu(max_k z_k + b): relu monotone, b per-feature.)
- Phase 2 for group g is emitted one group later (pooled is ready by then,
  so the in-order PE stream never stalls): 3 accumulating matmuls
  (src@wtop + pooled@wbot + ones33@bias33 for the per-column bias), ACT
  relu into pair staging. Its PSUM tile shares a 2-slot [128,512] ring
  with the DVE-drain tiles (8 PSUM banks total).
- DMA: neighbor blocks ramp small->large (128 KB..2 MB) on the sync HWDGE
  queue, prefetched ~4 groups ahead; group 0 uses graded A-tile widths
  (512/1024/1536) so the first drain fires ~10.8us in; w_pool/bias_pool
  ride the scalar HWDGE queue so the first block issues immediately;
  output stores are deferred until every block load is queued (a store
  waiting at the sync queue head would stall later loads), 2-group batched.
"""

import os

import numpy as np

N, K, DIM, OUT = 10000, 32, 128, 128
N_CORES = 8
NODES_PC = N // N_CORES          # 1250 nodes per core
ROWS_PC = NODES_PC * K           # 40000 neighbor rows per core

GROUP = 128                      # nodes per group
GCOLS = GROUP * K                # 4096 neighbor cols per full group
ATILE = 1536                     # ACT-drained PSUM tile (3 banks)
DTILE = 512                      # DVE-drained PSUM tile (1 bank)
MM = 512                         # cols per matmul (1 PSUM bank of fp32)
LAG = 1                          # groups between pooled ready and phase 2

# groups of (node0, n_nodes): 9 x 128 + 1 x 98
GROUPS = []
_n0 = 0
while _n0 < NODES_PC:
    _g = min(GROUP, NODES_PC - _n0)
    GROUPS.append((_n0, _g))
    _n0 += _g

# DMA blocks as col ranges: small leading blocks for a fast ramp
BLOCK_RANGES = [
    (0, 512), (512, 1536), (1536, 3072), (3072, 4096), (4096, 8192),
    (8192, 12288), (12288, 20480), (20480, 28672), (28672, 36864),
    (36864, 40000),
]


def _block_of_col(c):
    for b, (c0, c1) in enumerate(BLOCK_RANGES):
        if c0 <= c < c1:
            return b
    raise ValueError(c)


# first group that touches each block (for prefetch pacing)
FIRST_GROUP_OF_BLOCK = {}
for _g, (_n0, _gn) in enumerate(GROUPS):
    for _b in range(_block_of_col(_n0 * K), _block_of_col(_n0 * K + _gn * K - 1) + 1):
        FIRST_GROUP_OF_BLOCK.setdefault(_b, _g)

STRIP_LDW = os.environ.get("AGG_STRIP_LDW", "1") == "1"
# which engine applies the phase-2 relu: "act" or "dve"
P2_RELU = os.environ.get("AGG_P2_RELU", "act")


def _strip_redundant_ldweights(nc, mybir):
    """Remove LDWEIGHTS that reload the already-loaded stationary operand.

    Runs after Tile scheduling/legalization, so it sees the final per-engine
    instruction order. move_matmul_waits_to_ldweights first folds matmul
    waits onto the paired LDWEIGHTS; only signature-identical LDWEIGHTS that
    carry no waits/updates are dropped, so semaphore structure is preserved.
    """
    nc.move_matmul_waits_to_ldweights()
    stripped = 0
    for blk in nc.main_func.blocks:
        loaded = None
        keep = []
        for i in blk.instructions:
            if isinstance(i, mybir.InstLdweights):
                a = i.ins[0]
                sig = (
                    a.memref,
                    a.offset,
                    str(a.ap),
                    str(a.dtype),
                    i.is_transpose,
                    i.tile_size,
                    i.tile_position,
                    str(i.perf_mode),
                )
                si = i.sync_info
                clean = si is None or (not si.on_wait and not si.on_update)
                if clean and loaded == sig:
                    stripped += 1
                    continue
                loaded = sig
                keep.append(i)
                continue
            if isinstance(i, mybir.InstMatmult) and i.is_transpose:
                loaded = None
            keep.append(i)
        blk.instructions[:] = keep
    return stripped


def _build_nc():
    import concourse.bacc as bacc
    import concourse.mybir as mybir
    import concourse.tile as tile

    f32 = mybir.dt.float32
    bf16 = mybir.dt.bfloat16
    AX = mybir.AxisListType
    AF = mybir.ActivationFunctionType
    ALU = mybir.AluOpType

    nc = bacc.Bacc(target_bir_lowering=False)

    srcT = nc.declare_dram_parameter("srcT", [DIM, NODES_PC], bf16, isOutput=False)
    nbrT = nc.declare_dram_parameter("nbrT", [DIM, ROWS_PC], bf16, isOutput=False)
    w_pool = nc.declare_dram_parameter("w_pool", [DIM, DIM], bf16, isOutput=False)
    bias_pool = nc.declare_dram_parameter("bias_pool", [DIM], f32, isOutput=False)
    w = nc.declare_dram_parameter("w", [2 * DIM, OUT], bf16, isOutput=False)
    ones33 = nc.declare_dram_parameter("ones33", [32, GROUP], bf16, isOutput=False)
    bias33 = nc.declare_dram_parameter("bias33", [32, OUT], bf16, isOutput=False)
    out = nc.declare_dram_parameter("out", [NODES_PC, OUT], f32, isOutput=True)

    max_blk = max(c1 - c0 for c0, c1 in BLOCK_RANGES)

    with tile.TileContext(nc) as tc:
        with (
            tc.tile_pool(name="consts", bufs=1) as consts,
            tc.tile_pool(name="xt", bufs=6) as xt_pool,
            tc.tile_pool(name="hcp", bufs=3) as hcp_pool,
            tc.tile_pool(name="tree", bufs=3) as tree_pool,
            tc.tile_pool(name="pooled", bufs=4) as pooled_pool,
            tc.tile_pool(name="raw", bufs=3) as raw_pool,
            tc.tile_pool(name="outio", bufs=3) as outio,
            tc.tile_pool(name="ps_a", bufs=2, space="PSUM") as ps_a,
            tc.tile_pool(name="ps_m", bufs=2, space="PSUM") as ps_m,
        ):
            # --- w_pool + bias_pool first, then the first neighbor blocks,
            # then the phase-2-only constants (needed much later) ---
            wpool_sb = consts.tile([DIM, DIM], bf16)
            nc.scalar.dma_start(out=wpool_sb, in_=w_pool[:, :])
            bpool_sb = consts.tile([DIM, 1], f32)
            nc.scalar.dma_start(
                out=bpool_sb, in_=bias_pool.rearrange("(d o) -> d o", o=1)
            )
            zz = consts.tile([DIM, 64], bf16)
            nc.vector.memset(zz, 0.0)
            zcol = consts.tile([DIM, 1], f32)
            nc.vector.memset(zcol, 0.0)

            xt_tiles = [None] * len(BLOCK_RANGES)

            def load_block(bi):
                if bi >= len(BLOCK_RANGES) or xt_tiles[bi] is not None:
                    return
                c0, c1 = BLOCK_RANGES[bi]
                xT = xt_pool.tile([DIM, max_blk], bf16, tag="xT", name="xT")
                nc.sync.dma_start(out=xT[:, : c1 - c0], in_=nbrT[:, c0:c1])
                xt_tiles[bi] = xT

            for _b in range(6):
                load_block(_b)

            wtop_sb = consts.tile([DIM, OUT], bf16)
            nc.sync.dma_start(out=wtop_sb, in_=w[0:DIM, :])
            wbot_sb = consts.tile([DIM, OUT], bf16)
            nc.sync.dma_start(out=wbot_sb, in_=w[DIM : 2 * DIM, :])
            ones_sb = consts.tile([32, GROUP], bf16)
            nc.sync.dma_start(out=ones_sb, in_=ones33[:, :])
            bias33_sb = consts.tile([32, OUT], bf16)
            nc.sync.dma_start(out=bias33_sb, in_=bias33[:, :])
            srcT_sb = consts.tile([DIM, NODES_PC], bf16)
            nc.sync.dma_start(out=srcT_sb, in_=srcT[:, :])

            pooled_tiles = [None] * len(GROUPS)
            pair_tiles = {}
            pending_stores = []

            def maybe_flush_stores(force=False):
                # stores ride the sync queue; only emit once every block
                # load is already queued so they can never delay a load
                if not force and any(t is None for t in xt_tiles):
                    return
                while pending_stores:
                    o_dst, o_src = pending_stores.pop(0)
                    nc.sync.dma_start(out=o_dst, in_=o_src)

            def emit_phase2(g):
                n0, gn = GROUPS[g]
                pooled = pooled_tiles[g]
                o_ps = ps_m.tile([DIM, MM], f32, tag="hd", name="o_ps")
                nc.tensor.matmul(
                    out=o_ps[:gn, :OUT],
                    lhsT=srcT_sb[:, n0 : n0 + gn],
                    rhs=wtop_sb[:, :],
                    start=True,
                    stop=False,
                )
                nc.tensor.matmul(
                    out=o_ps[:gn, :OUT],
                    lhsT=pooled[:, :gn],
                    rhs=wbot_sb[:, :],
                    start=False,
                    stop=False,
                )
                nc.tensor.matmul(
                    out=o_ps[:gn, :OUT],
                    lhsT=ones_sb[:, :gn],
                    rhs=bias33_sb[:, :],
                    start=False,
                    stop=True,
                )
                pi = g // 2
                if pi not in pair_tiles:
                    pair_tiles[pi] = outio.tile(
                        [GROUP, 2, OUT], f32, tag="opair", name="opair"
                    )
                o_pair = pair_tiles[pi]
                dst = o_pair[:gn, g % 2, :]
                use_act = (g % 2 == 0) if P2_RELU == "alt" else (P2_RELU == "act")
                if use_act:
                    # bias as an SBUF AP: a float bias would pull in the
                    # const-AP table and its per-engine preamble TENSOR_LOAD
                    nc.scalar.activation(
                        out=dst, in_=o_ps[:gn, :OUT], func=AF.Relu,
                        bias=zcol[:gn, :],
                    )
                else:
                    nc.vector.tensor_scalar_max(
                        out=dst, in0=o_ps[:gn, :OUT], scalar1=0.0
                    )
                # store when the pair is complete (or at the very end)
                if g % 2 == 1 and gn == GROUP:
                    pending_stores.append(
                        (
                            out[n0 - GROUP : n0 + gn].rearrange(
                                "(q p) o -> p q o", p=GROUP
                            ),
                            o_pair[:, 0:2, :],
                        )
                    )
                elif g % 2 == 1:
                    pending_stores.append((out[n0 - GROUP : n0], o_pair[:, 0, :]))
                    pending_stores.append((out[n0 : n0 + gn], dst))
                elif g == len(GROUPS) - 1:
                    pending_stores.append((out[n0 : n0 + gn], dst))
                maybe_flush_stores()

            for gi, (n0, gn) in enumerate(GROUPS):
                c0 = n0 * K
                gc = gn * K
                for b, fg in FIRST_GROUP_OF_BLOCK.items():
                    if fg <= gi + 4:
                        load_block(b)

                def xsrc(col):
                    b = _block_of_col(col)
                    return xt_tiles[b], col - BLOCK_RANGES[b][0]

                # split: leading cols -> ACT, trailing 512s -> DVE. Group 0
                # uses graded tile widths so the first drain starts ASAP.
                a_widths = [512, 1024, ATILE] if gi == 0 else [ATILE, ATILE]
                a_cols = sum(a_widths)
                a_nodes = a_cols // K
                d_nodes = gn - a_nodes

                hcp = hcp_pool.tile([DIM, 3 * ATILE], bf16, tag="hcp", name="hcp")
                raw = raw_pool.tile([DIM, 64], f32, tag="raw", name="raw")
                pooled = pooled_pool.tile(
                    [DIM, GROUP], bf16, tag="pooled", name="pooled"
                )
                pooled_tiles[gi] = pooled

                # --- ACT-drained tiles ---
                t0 = 0
                for tw_a in a_widths:
                    hT = ps_a.tile([DIM, ATILE], f32, tag="hA", name="hA")
                    for m0 in range(0, tw_a, MM):
                        xS, xo = xsrc(c0 + t0 + m0)
                        nc.tensor.matmul(
                            out=hT[:, m0 : m0 + MM],
                            lhsT=wpool_sb[:, :],
                            rhs=xS[:, xo : xo + MM],
                            start=True,
                            stop=True,
                        )
                    nc.scalar.activation(
                        out=hcp[:, t0 : t0 + tw_a],
                        in_=hT[:, :tw_a],
                        func=AF.Relu,
                        bias=bpool_sb[:, :],
                        scale=1.0,
                    )
                    t0 += tw_a

                # --- DVE-drained tiles ---
                t0 = a_cols
                while t0 < gc:
                    tw = min(DTILE, gc - t0)
                    hD = ps_m.tile([DIM, DTILE], f32, tag="hd", name="hD")
                    xS, xo = xsrc(c0 + t0)
                    nc.tensor.matmul(
                        out=hD[:, :tw],
                        lhsT=wpool_sb[:, :],
                        rhs=xS[:, xo : xo + tw],
                        start=True,
                        stop=True,
                    )
                    rn0 = (t0 - a_cols) // K
                    nc.vector.reduce_max(
                        out=raw[:, rn0 : rn0 + tw // K],
                        in_=hD[:, :tw].rearrange("p (n k) -> p n k", k=K),
                        axis=AX.X,
                    )
                    t0 += tw

                # --- finish: DVE-side bias+relu, ACT-side bf16 max tree ---
                nc.vector.scalar_tensor_tensor(
                    out=pooled[:, a_nodes:gn],
                    in0=raw[:, :d_nodes],
                    scalar=bpool_sb[:, :],
                    in1=zz[:, :d_nodes],
                    op0=ALU.add,
                    op1=ALU.max,
                )

                m = a_nodes
                tA = tree_pool.tile([DIM, 96 * 16], bf16, tag="tA", name="tA")
                tB = tree_pool.tile([DIM, 96 * 8], bf16, tag="tB", name="tB")
                tC = tree_pool.tile([DIM, 96 * 4], bf16, tag="tC", name="tC")
                tD = tree_pool.tile([DIM, 96 * 2], bf16, tag="tD", name="tD")
                v = hcp[:, :a_cols].rearrange("p (n k) -> p n k", k=K)
                a = tA[:, : m * 16].rearrange("p (n j) -> p n j", j=16)
                nc.vector.tensor_max(out=a, in0=v[:, :, 0:16], in1=v[:, :, 16:32])
                b = tB[:, : m * 8].rearrange("p (n j) -> p n j", j=8)
                nc.vector.tensor_max(out=b, in0=a[:, :, 0:8], in1=a[:, :, 8:16])
                cc = tC[:, : m * 4].rearrange("p (n j) -> p n j", j=4)
                nc.vector.tensor_max(out=cc, in0=b[:, :, 0:4], in1=b[:, :, 4:8])
                d = tD[:, : m * 2].rearrange("p (n j) -> p n j", j=2)
                nc.vector.tensor_max(out=d, in0=cc[:, :, 0:2], in1=cc[:, :, 2:4])
                p1 = pooled[:, :m].rearrange("p (n j) -> p n j", j=1)
                nc.vector.tensor_max(out=p1, in0=d[:, :, 0:1], in1=d[:, :, 1:2])

                if gi >= LAG:
                    emit_phase2(gi - LAG)

            for g in range(len(GROUPS) - LAG, len(GROUPS)):
                emit_phase2(g)
            maybe_flush_stores(force=True)

    if STRIP_LDW:
        _strip_redundant_ldweights(nc, mybir)
    nc.compile()
    return nc


def _make_in_maps(inputs):
    import ml_dtypes

    bf = ml_dtypes.bfloat16
    src = np.asarray(inputs["src_features"], dtype=np.float32)
    nbr = np.asarray(inputs["neighbor_features"], dtype=np.float32)
    w_pool = np.asarray(inputs["w_pool"], dtype=np.float32).astype(bf)
    bias_pool = np.ascontiguousarray(inputs["bias_pool"], dtype=np.float32)
    w = np.asarray(inputs["w"], dtype=np.float32).astype(bf)
    bias = np.asarray(inputs["bias"], dtype=np.float32)

    ones33 = np.zeros((32, GROUP), dtype=np.float32)
    ones33[0, :] = 1.0
    bias33 = np.zeros((32, OUT), dtype=np.float32)
    bias33[0, :] = bias
    ones33 = ones33.astype(bf)
    bias33 = bias33.astype(bf)

    in_maps = []
    for c in range(N_CORES):
        in_maps.append(
            {
                "srcT": np.ascontiguousarray(
                    src[c * NODES_PC : (c + 1) * NODES_PC].T.astype(bf)
                ),
                "nbrT": np.ascontiguousarray(
                    nbr[c * ROWS_PC : (c + 1) * ROWS_PC].T.astype(bf)
                ),
                "w_pool": w_pool,
                "bias_pool": bias_pool,
                "w": w,
                "ones33": ones33,
                "bias33": bias33,
            }
        )
    return in_maps


_NC_CACHE = None


def kernel(**inputs: np.ndarray) -> np.ndarray:
    from concourse.bass_utils import run_bass_kernel_spmd

    global _NC_CACHE
    if _NC_CACHE is None:
        _NC_CACHE = _build_nc()
    nc = _NC_CACHE

    in_maps = _make_in_maps(inputs)
    res = run_bass_kernel_spmd(nc, in_maps, core_ids=list(range(N_CORES)))
    return np.concatenate([res.results[c]["out"] for c in range(N_CORES)], axis=0)
